# revision 39
# baseline (speedup 1.0000x reference)
"""GQA attention (RoPE, causal, per-head q-scale) on 8 TRN2 NeuronCores.

Sharding: 2-way data-parallel over batch x 4-way tensor-parallel over heads.
Core c handles batch b=c//4 and head group g=c%4 (8 q heads, 2 kv heads).
Each core computes qkv-proj -> rope -> causal attention -> partial o_proj
(over its heads' columns of Wo); the host sums the 4 partials per batch.

All scalar factors (rope_mscale, sm_scale, per_head_scale) are folded into
the Wq/Wk rows on the host. Causal masking: fully-masked column blocks are
skipped (matmul widths trimmed to the causal extent); diagonal blocks are
masked by multiplying the exp'd bf16 tile with a 0/1 lower-triangular
constant on the vector engine (the Vaug ones-columns then produce exact
softmax denominators).

dtypes: matmuls run in bf16 with f32 PSUM accumulation; rope runs in bf16
(inputs are bf16 for the scores matmul anyway); softmax normalization is
f32 (reciprocal_approx_fast). The o_proj partials are written out in bf16
(summed in f64 on the host).

Emission is software-pipelined at sk-block granularity: background quanta
(QKV(j+1) k-passes, o_proj(j-1) column blocks) are interleaved between
attention score blocks so the PE always has exp-independent work while the
scalar engine streams the exps. The j=0 QKV runs as a DMA-paced prologue:
per-k passes for all 6 output chunks are interleaved with the streaming
weight/activation loads, so real PE work starts ~2us into the kernel.

Layouts on device (partition, free):
  xt      [hid, s]        hidden^T, streamed in 512-col chunks
  wqkv    [hid, 768]      [Wq(8 heads, scaled) | Wk(2 kv, scaled) | Wv].T
  q/k^T   [d*heads, s]    head-major rows; rope applied in this layout
  scores^T[sk, sq]        per (head-pair, sk-chunk 128, sq-chunk 512) in
                          PSUM; band blocks compacted: head A in cols
                          [off:512], head B in [512:1024-off]
  exp^T   [sk, sq]        SBUF bf16, fed as matmul rhs
  Vaug    [sk, 128]       V rows (0:64) + 64 ones cols; PV matmul output
                          rows 64:128 then hold the softmax denominators
                          already broadcast over 64 partitions
  out^T   [2d, sq]        PSUM accumulator per (head, sq-chunk)
  attn^T  [o(=2 heads), s] normalized bf16, lhsT for o_proj
  out     [s, hid_out]    partial o_proj result (bf16), one per core
"""

import sys, os
from collections import deque

for _p in ("/opt/trn_rl_repo", "/root/.axon_site/_ro/trn_rl_repo"):
    if os.path.isdir(_p) and _p not in sys.path:
        sys.path.insert(0, _p)

import numpy as np

import concourse.bass as bass
import concourse.mybir as mybir
import concourse.tile as tile
from concourse import bacc
from concourse.bass_utils import run_bass_kernel_spmd

F32 = mybir.dt.float32
BF16 = mybir.dt.bfloat16
AF = mybir.ActivationFunctionType

B, S, HID = 2, 2048, 2048
H, K, D = 32, 8, 64
G = H // K
ROPE_MSCALE = 1.2
SM_SCALE = 1.0 / (D ** 0.5)

NH = 8           # q heads per core
NKV = 2          # kv heads per core
NPAIR = 4        # q head pairs per core
QO = NH * D      # 512 q rows
NK = HID // 128  # 16 contraction chunks
SQW = 512        # sq / xt chunk width
NJ = S // SQW    # 4 chunks
NSK = S // 128   # 16 sk chunks

_CACHED = {}


def _build():
    if "nc" in _CACHED:
        return _CACHED["nc"]

    nc = bacc.Bacc(None)

    xt_d = nc.declare_dram_parameter("xt", [HID, S], BF16, isOutput=False)
    wqkv_d = nc.declare_dram_parameter("wqkv", [HID, 768], BF16, isOutput=False)
    wo_d = nc.declare_dram_parameter("wo", [QO, HID], BF16, isOutput=False)
    cost_d = nc.declare_dram_parameter("cost", [128, S], BF16, isOutput=False)
    sints_d = nc.declare_dram_parameter("sints", [64, S], BF16, isOutput=False)
    constsb_d = nc.declare_dram_parameter("constsb", [128, 256], BF16, isOutput=False)
    out_d = nc.declare_dram_parameter("out", [S, HID], BF16, isOutput=True)

    with tile.TileContext(nc) as tc:
        # ---------- long-lived pools ----------
        with (
            tc.tile_pool(name="consts", bufs=1) as consts_pool,
            tc.tile_pool(name="ktv", bufs=1) as ktv_pool,
            tc.tile_pool(name="qrope", bufs=10) as qrope_pool,
            tc.tile_pool(name="expt", bufs=8) as expt_pool,
            tc.tile_pool(name="attnt", bufs=8) as attnt_pool,
            tc.tile_pool(name="inv", bufs=2) as inv_pool,
            tc.tile_pool(name="wo", bufs=1) as wo_pool,
            tc.tile_pool(name="ost", bufs=4) as ost_pool,
            tc.tile_pool(name="wq", bufs=1) as wq_pool,
            tc.tile_pool(name="xt", bufs=12) as xt_pool,
            tc.tile_pool(name="cs", bufs=1) as cs_pool,
            tc.tile_pool(name="rtmp", bufs=2) as rtmp_pool,
            tc.tile_pool(name="psc", bufs=2, space="PSUM") as psc_pool,
            tc.tile_pool(name="pout2", bufs=2, space="PSUM") as pout2_pool,
            tc.tile_pool(name="pqkv", bufs=2, space="PSUM") as pqkv_pool,
        ):
            xt_r = xt_d.rearrange("(kc p) s -> p kc s", p=128)
            xt_tiles = {}

            def load_xt_kq(j, kq):
                if j not in xt_tiles:
                    xt_tiles[j] = [None] * 4
                tt = xt_pool.tile([128, 4, SQW], BF16, tag="xt", name="xtt")
                nc.sync.dma_start(
                    out=tt,
                    in_=xt_r[:, kq * 4:(kq + 1) * 4, j * SQW:(j + 1) * SQW],
                )
                xt_tiles[j][kq] = tt

            def load_xt(j):
                for kq in range(4):
                    load_xt_kq(j, kq)

            # tiny HAM warm-up: a few dummy matmuls ramp the PE clock while
            # the first weight/activation slices stream in.
            dummy = cs_pool.tile([128, 512], BF16, tag="dummy", name="dummy")
            nc.vector.memset(dummy, 1.0)
            pwarm = pout2_pool.tile([128, 512], F32, tag="p2", name="pwarm")
            for _ in range(4):
                nc.tensor.matmul(
                    pwarm, dummy[:, 0:128], dummy, start=True, stop=True,
                    skip_group_check=True,
                )

            # interleaved DMA issue: the host lays wqkv out as
            # [Wk | Wq-pair0 | Wv | Wq-pairs1-3] so the prologue-critical
            # first 384 cols stream as one contiguous run per k-slice,
            # arriving just before the prologue's k-pass needs them; the
            # remaining q-pair weights stream second.
            wqt = wq_pool.tile([128, NK, 768], BF16, name="wqt")
            def wq_col(m):
                # host layout [Wk | Wq-pair0 | Wv | Wq-pair1..3]
                if m == 4:
                    return 0        # k-proj
                if m == 0:
                    return 128      # q pair 0
                if m == 5:
                    return 256      # v-proj
                return 256 + 128 * m  # q pairs 1..3 -> 384/512/640

            wq_r = wqkv_d.rearrange("(kc p) c -> p kc c", p=128)
            costd = cs_pool.tile([128, S], BF16, tag="cost", name="costd")
            sints = cs_pool.tile([64, S], BF16, tag="sints", name="sints")
            constsb = consts_pool.tile([128, 256], BF16, name="constsb")

            # the sync queue's in-flight cap self-throttles: transfers land
            # in issue order at fabric rate, so the critical stream (k, V,
            # q-pairs 0-1, rope tables) is issued first in consumption
            # order with chunked transfers (per-transfer trigger
            # instructions serialize on the issuing engine, so fewer,
            # bigger DMAs issue faster), and the secondary stream strictly
            # after
            nc.sync.dma_start(out=constsb, in_=constsb_d[:, :])
            load_xt_kq(0, 0)
            nc.sync.dma_start(out=wqt[:, 0:4, 0:384], in_=wq_r[:, 0:4, 0:384])
            load_xt_kq(0, 1)
            nc.sync.dma_start(out=wqt[:, 4:8, 0:384], in_=wq_r[:, 4:8, 0:384])
            load_xt_kq(0, 2)
            nc.sync.dma_start(out=wqt[:, 8:12, 0:384], in_=wq_r[:, 8:12, 0:384])
            load_xt_kq(0, 3)
            nc.sync.dma_start(out=wqt[:, 12:16, 0:384], in_=wq_r[:, 12:16, 0:384])
            nc.sync.dma_start(out=sints, in_=sints_d[:, :])
            nc.sync.dma_start(out=costd, in_=cost_d[:, :])
            # q-pair 1 weights, then the long tail: q-pairs 2-3, xt(1), Wo
            nc.sync.dma_start(out=wqt[:, :, 384:512], in_=wq_r[:, :, 384:512])
            nc.sync.dma_start(out=wqt[:, :, 512:768], in_=wq_r[:, :, 512:768])
            wot = wo_pool.tile([128, NPAIR, HID], BF16, name="wot")
            wo_r = wo_d.rearrange("(m p) h -> p m h", p=128)
            xt_tiles[1] = [None] * 4
            for kq in range(4):
                tt = xt_pool.tile([128, 4, SQW], BF16, tag="xt", name="xtt")
                nc.sync.dma_start(
                    out=tt, in_=xt_r[:, kq * 4:(kq + 1) * 4, SQW:2 * SQW]
                )
                xt_tiles[1][kq] = tt
            nc.sync.dma_start(out=wot, in_=wo_r[:, :, :])

            mask01 = constsb[:, 0:128]      # 1.0 where sq >= sk (in-block)
            ones_colb = constsb[:, 128:129]

            kt_aa = ktv_pool.tile([128, S], BF16, tag="ktaa", name="ktaa")
            kt_bb = ktv_pool.tile([128, S], BF16, tag="ktbb", name="ktbb")
            # Vaug: cols 0:64 = V, cols 64:128 = 1.0 (sums -> rows 64:128)
            vaug = [
                ktv_pool.tile([128, NSK, 128], BF16, tag=f"vaug{i}", name=f"vaug{i}")
                for i in range(NKV)
            ]

            qrope = {}   # (m, j) -> tile [128, SQW] bf16
            attnt = {}   # (m, j) -> tile [128, SQW] bf16

            def rope(psum_q, j, dst):
                """RoPE a [128, SQW] projected chunk (2 heads) into dst.
                q' = q*cos + swap_halves(q)*sin_signed.  The five multiplies
                read PSUM directly (mixed-space ops may differ in base
                partition; SBUF-SBUF ops may not) and emit bf16, so the
                final add runs as a 2-byte SBUF op in the DVE fast mode."""
                c0, c1 = j * SQW, (j + 1) * SQW
                t2 = rtmp_pool.tile([128, SQW], BF16, tag="t2", name="t2")
                for base in (0, 64):
                    nc.vector.tensor_mul(
                        t2[base:base + 32, :], psum_q[base + 32:base + 64, :],
                        sints[0:32, c0:c1],
                    )
                    nc.vector.tensor_mul(
                        t2[base + 32:base + 64, :], psum_q[base:base + 32, :],
                        sints[32:64, c0:c1],
                    )
                t4 = rtmp_pool.tile([128, SQW], BF16, tag="t4", name="t4")
                nc.vector.tensor_mul(t4, psum_q, costd[:, c0:c1])
                nc.vector.tensor_add(dst, t2, t4)

            def finish_k_chunk(pq, j):
                kro = rtmp_pool.tile([128, SQW], BF16, tag="kro", name="kro")
                rope(pq, j, kro)
                c0, c1 = j * SQW, (j + 1) * SQW
                # partition-aligned copies on the scalar engine, shifted
                # ones on the vector engine: halves the serialized chain
                nc.scalar.copy(kt_aa[0:64, c0:c1], kro[0:64, :])
                nc.vector.tensor_copy(kt_aa[64:128, c0:c1], kro[0:64, :])
                nc.vector.tensor_copy(kt_bb[0:64, c0:c1], kro[64:128, :])
                nc.scalar.copy(kt_bb[64:128, c0:c1], kro[64:128, :])

            # ---------- background work quanta ----------
            bg = deque()  # (tag, fn)
            bg_state = {"credit": 0.0, "rate": 0.0}

            def bg_set_rate(blocks):
                bg_state["rate"] = len(bg) / max(blocks, 1)
                bg_state["credit"] = 0.0

            def bg_tick():
                bg_state["credit"] += bg_state["rate"]
                while bg_state["credit"] >= 1.0 and bg:
                    bg.popleft()[1]()
                    bg_state["credit"] -= 1.0

            def bg_flush():
                while bg:
                    bg.popleft()[1]()

            def bg_drain_qkv():
                while any(t == "qkv" for t, _ in bg):
                    bg.popleft()[1]()

            def qkv_quanta(j, m):
                """qkv chunk m of sq-chunk j as 4 quanta of 4 k-passes."""
                state = {}

                def passes(k0):
                    def f():
                        if k0 == 0:
                            state["pq"] = pqkv_pool.tile(
                                [128, SQW], F32, tag="qkv", name="pqkv"
                            )
                        pq = state["pq"]
                        wc = wq_col(m)
                        for k in range(k0, k0 + 4):
                            nc.tensor.matmul(
                                pq,
                                wqt[:, k, wc:wc + 128],
                                xt_tiles[j][k // 4][:, k % 4, :],
                                start=(k == 0),
                                stop=(k == NK - 1),
                            )
                        if k0 == 12:
                            if m < NPAIR:
                                qrope[(m, j)] = qrope_pool.tile(
                                    [128, SQW], BF16, tag="qr", name="qr"
                                )
                                rope(pq, j, qrope[(m, j)])
                            else:
                                finish_k_chunk(pq, j)
                    return f

                return [("qkv", passes(0)), ("qkv", passes(4)),
                        ("qkv", passes(8)), ("qkv", passes(12))]

            def v_quanta(j, copy_eng=None, h2s=(0, 1, 2, 3)):
                """V chunk of sq-chunk j as quanta (one per 128 tokens)."""
                def one(h2):
                    def f():
                        sk = (j * SQW) // 128 + h2
                        pv = pqkv_pool.tile(
                            [128, 128], F32, tag="qkv", name="pv"
                        )
                        for k in range(NK):
                            nc.tensor.matmul(
                                pv,
                                xt_tiles[j][k // 4][:, k % 4, h2 * 128:(h2 + 1) * 128],
                                wqt[:, k, 256:384],
                                start=(k == 0),
                                stop=(k == NK - 1),
                            )
                        for i in range(NKV):
                            if copy_eng is nc.scalar:
                                nc.scalar.copy(
                                    vaug[i][:, sk, 0:64],
                                    pv[:, i * 64:(i + 1) * 64],
                                )
                            else:
                                nc.vector.tensor_copy(
                                    vaug[i][:, sk, 0:64],
                                    pv[:, i * 64:(i + 1) * 64],
                                )
                    return f

                return [("qkv", one(h2)) for h2 in h2s]

            _ocp = {"n": 0}

            def oproj_quanta(j, pools=None):
                """o_proj of sq-chunk j: 16 quanta (one per 128 tokens x
                512 hid cols).  Copies rotate over vector/scalar/gpsimd;
                the final chunk also rotates psum pools so the tail is not
                serialized on two banks."""
                pools = pools or [(pqkv_pool, "qkv")]

                ots = {}

                def one(sc, hc):
                    def f():
                        i = _ocp["n"]
                        _ocp["n"] += 1
                        pool, ptag = pools[i % len(pools)]
                        po = pool.tile([128, 512], F32, tag=ptag, name="po")
                        for m in range(NPAIR):
                            nc.tensor.matmul(
                                po,
                                attnt[(m, j)][:, sc * 128:(sc + 1) * 128],
                                wot[:, m, hc * 512:(hc + 1) * 512],
                                start=(m == 0),
                                stop=(m == NPAIR - 1),
                            )
                        # gather the four 512-col blocks of this 128-token
                        # group into one tile: a single contiguous-row DMA
                        # moves 4KB/row instead of 4 fragmented 1KB writes
                        if hc == 0:
                            ots[sc] = ost_pool.tile(
                                [128, HID], BF16, tag="ot", name="ot"
                            )
                        ot = ots[sc]
                        if i % 2 == 0:
                            nc.vector.tensor_copy(
                                ot[:, hc * 512:(hc + 1) * 512], po
                            )
                        else:
                            nc.scalar.copy(ot[:, hc * 512:(hc + 1) * 512], po)
                        if hc == HID // 512 - 1:
                            r0 = j * SQW + sc * 128
                            nc.gpsimd.dma_start(
                                out=out_d[r0:r0 + 128, :], in_=ots.pop(sc)
                            )
                        if sc == SQW // 128 - 1 and hc == HID // 512 - 1:
                            for m in range(NPAIR):
                                attnt.pop((m, j))
                    return f

                return [("oproj", one(sc, hc)) for sc in range(4) for hc in range(4)]

            _done_pairs = set()
            _fin = {"f": None}  # deferred tail of the previous pair

            def attention_pair(j, m):
                if (j, m) in _done_pairs:
                    return
                _done_pairs.add((j, m))
                nsk = 4 * (j + 1)
                kt = kt_aa if m < 2 else kt_bb
                va = vaug[m // 2]
                qr = qrope.pop((m, j))
                p2 = {}
                for hb in (0, 64):  # head A at 0, head B at 64
                    p2[hb] = pout2_pool.tile(
                        [128, SQW], F32, tag="p2", name="p2"
                    )
                pend = []  # staged (exp tile, sk)

                def pv_step():
                    et2, psk = pend.pop(0)
                    poff = (psk - 4 * j) * 128 if psk >= 4 * j else 0
                    nc.tensor.matmul(
                        p2[0][:, poff:SQW],
                        va[:, psk, :],
                        et2[:, poff:SQW],
                        start=(psk == 0),
                        stop=(psk == nsk - 1),
                        skip_group_check=True,
                    )
                    nc.tensor.matmul(
                        p2[64][:, poff:SQW],
                        va[:, psk, :],
                        et2[:, SQW:2 * SQW - poff],
                        start=(psk == 0),
                        stop=(psk == nsk - 1),
                        skip_group_check=True,
                    )

                for sk in range(nsk):
                    # both heads' scores in one 2-bank tile; band blocks
                    # compacted so the valid region is contiguous:
                    # head A [off:512], head B [512:1024-off]
                    p1 = psc_pool.tile([128, 2 * SQW], F32, tag="sc", name="sc")
                    band = sk >= 4 * j
                    off = (sk - 4 * j) * 128 if band else 0
                    nc.tensor.matmul(
                        p1[:, off:SQW],
                        kt[0:64, sk * 128:(sk + 1) * 128],
                        qr[0:64, off:SQW],
                        start=True, stop=True, skip_group_check=True,
                    )
                    nc.tensor.matmul(
                        p1[:, SQW:2 * SQW - off],
                        kt[64:128, sk * 128:(sk + 1) * 128],
                        qr[64:128, off:SQW],
                        start=True, stop=True, skip_group_check=True,
                    )
                    et = expt_pool.tile(
                        [128, 2 * SQW], BF16, tag="et", name="et"
                    )
                    nc.scalar.activation(
                        et[:, off:2 * SQW - off], p1[:, off:2 * SQW - off],
                        AF.Exp,
                    )
                    if band:
                        # causal mask on the diagonal 128x128 sub-blocks,
                        # on the (otherwise idle) gpsimd engine so the
                        # exp->mask->PV chain skips the deep DVE queue
                        nc.gpsimd.tensor_mul(
                            et[:, off:off + 128], et[:, off:off + 128], mask01
                        )
                        nc.gpsimd.tensor_mul(
                            et[:, SQW:SQW + 128], et[:, SQW:SQW + 128], mask01
                        )
                    pend.append((et, sk))
                    if sk == 1 and _fin["f"] is not None:
                        # emit the previous pair's deferred tail now: its
                        # last exps drain while this pair's first scores
                        # run, instead of stalling the PE at the boundary
                        _fin["f"]()
                        _fin["f"] = None
                    bg_tick()
                    while len(pend) > 3:
                        pv_step()

                def finish():
                    while pend:
                        pv_step()
                    # normalize: attnT = out^T * (1/sums); sums come out
                    # of the PV matmul pre-broadcast in psum rows 64:128
                    at = attnt_pool.tile([128, SQW], BF16, tag="at", name="at")
                    for hb in (0, 64):
                        sums = inv_pool.tile(
                            [64, SQW], F32, tag="sums", name="sums"
                        )
                        nc.vector.tensor_copy(sums, p2[hb][64:128, :])
                        invb = inv_pool.tile(
                            [64, SQW], F32, tag="invb", name="invb"
                        )
                        nc.vector.reciprocal_approx_fast(out=invb, in_=sums)
                        nc.vector.tensor_mul(
                            at[hb:hb + 64, :], p2[hb][0:64, :], invb
                        )
                    attnt[(m, j)] = at

                if _fin["f"] is not None:
                    # nsk < 2 never happens, but guard double-defer anyway
                    _fin["f"]()
                _fin["f"] = finish

            # ---------- j=0 QKV prologue, DMA-paced ----------
            # Critical path only: k-chunk + q-pair 0 interleaved per-k (the
            # PE consumes weight slices as they stream in), then V with one
            # psum accumulation group open at a time.  q-pairs 1-3 are
            # seeded as background quanta and run during attention(0, *).
            pk0 = pqkv_pool.tile([128, SQW], F32, tag="qkv", name="pk0")
            pm0 = pqkv_pool.tile([128, SQW], F32, tag="qkv", name="pm0")
            # V token-groups 0-1 ride the (idle until attention) psc slots
            # so they stream through the k-loop with everything else
            pv01 = [
                psc_pool.tile([128, 2 * SQW], F32, tag="sc", name=f"pvh{h2}")
                for h2 in (0, 1)
            ]
            for k in range(NK):
                xt_t0 = xt_tiles[0][k // 4][:, k % 4, :]
                st, sp = (k == 0), (k == NK - 1)
                nc.tensor.matmul(pk0, wqt[:, k, 0:128], xt_t0, start=st, stop=sp)
                nc.tensor.matmul(pm0, wqt[:, k, 128:256], xt_t0, start=st, stop=sp)
                for h2 in (0, 1):
                    nc.tensor.matmul(
                        pv01[h2][:, 0:128],
                        xt_tiles[0][k // 4][:, k % 4, h2 * 128:(h2 + 1) * 128],
                        wqt[:, k, 256:384],
                        start=st, stop=sp,
                    )
            finish_k_chunk(pk0, 0)
            qrope[(0, 0)] = qrope_pool.tile([128, SQW], BF16, tag="qr", name="qr")
            rope(pm0, 0, qrope[(0, 0)])
            # ones-fill emitted here so it never blocks the rope chain in
            # the DVE queue (constsb is loaded first and long since landed)
            for i in range(NKV):
                nc.vector.tensor_copy(
                    vaug[i][:, :, 64:128],
                    ones_colb[:, None, :].broadcast_to([128, NSK, 64]),
                )
            for h2 in (0, 1):
                for i in range(NKV):
                    nc.scalar.copy(
                        vaug[i][:, h2, 0:64],
                        pv01[h2][:, i * 64:(i + 1) * 64],
                    )
            for _, f in v_quanta(0, copy_eng=nc.scalar, h2s=(2, 3)):
                f()
            for m in range(1, NPAIR):
                bg.extend(qkv_quanta(0, m))
            # q-pairs 1-3 keep the PE fed while the rope/copy chain
            # drains on DVE/ACT
            for _ in range(12):
                bg.popleft()[1]()

            def ensure_qrope(j, m):
                if (j, m) in _done_pairs:
                    return
                while (m, j) not in qrope and bg:
                    bg.popleft()[1]()

            # ---------- main loop ----------
            for j in range(NJ):
                if j > 0:
                    xt_tiles.pop(j - 1)
                if j + 2 < NJ:
                    load_xt(j + 2)
                if j + 1 < NJ:
                    for m in range(5):
                        bg.extend(qkv_quanta(j + 1, m))
                    bg.extend(v_quanta(j + 1))
                if j > 0:
                    bg.extend(oproj_quanta(j - 1))
                nsk = 4 * (j + 1)
                blocks = 4 * nsk + (2 * 4 * NJ if j == NJ - 2 else 0)
                if j == NJ - 1:
                    blocks = 2 * nsk
                bg_set_rate(blocks)
                for m in range(NPAIR):
                    ensure_qrope(j, m)
                    attention_pair(j, m)
                if j == NJ - 2:
                    # pull two of the last chunk's head-pairs forward so the
                    # tail window keeps the PE fed; their kt/vaug/qrope
                    # writes must be emitted first
                    bg_drain_qkv()
                    ensure_qrope(NJ - 1, 0)
                    attention_pair(NJ - 1, 0)
                    ensure_qrope(NJ - 1, 1)
                    attention_pair(NJ - 1, 1)
                bg_flush()
            if _fin["f"] is not None:
                _fin["f"]()
                _fin["f"] = None
            for _, f in oproj_quanta(
                NJ - 1,
                pools=[(pqkv_pool, "qkv"), (psc_pool, "sc"), (pout2_pool, "p2")],
            ):
                f()

    nc.finalize()
    _CACHED["nc"] = nc
    return nc


def _prep_inputs(cos, sin, hidden_states, per_head_scale, Wqkv, Wo):
    """Build the 8 per-core input maps (host-side, free)."""
    import ml_dtypes
    cos = np.asarray(cos, np.float32)
    sin = np.asarray(sin, np.float32)
    hs = np.asarray(hidden_states, np.float32)
    phs = np.asarray(per_head_scale, np.float32)
    Wqkv = np.asarray(Wqkv, np.float32)
    Wo = np.asarray(Wo, np.float32)

    cost = np.ascontiguousarray(np.vstack([cos.T, cos.T])).astype(ml_dtypes.bfloat16)
    st = sin.T.copy()
    st[0:32] *= -1.0
    sints = np.ascontiguousarray(st).astype(ml_dtypes.bfloat16)

    mask01 = np.zeros((128, 128), np.float32)
    for p in range(128):
        mask01[p, p:] = 1.0
    pad = np.zeros((128, 128), np.float32)
    pad[:, 0] = 1.0
    constsb = np.ascontiguousarray(
        np.concatenate([mask01, pad], axis=1)
    ).astype(ml_dtypes.bfloat16)

    xt_b = [np.ascontiguousarray(hs[b].T).astype(ml_dtypes.bfloat16) for b in range(B)]

    in_maps = []
    for c in range(8):
        b, g = c // 4, c % 4
        hq0 = NH * g
        wq = Wqkv[hq0 * D:(hq0 + NH) * D, :].copy()
        for h in range(NH):
            wq[h * D:(h + 1) * D] *= (
                ROPE_MSCALE * SM_SCALE * phs[b, hq0 + h]
            )
        kv0 = H * D + NKV * g * D
        wk = Wqkv[kv0:kv0 + NKV * D, :] * ROPE_MSCALE
        v0 = (H + K) * D + NKV * g * D
        wv = Wqkv[v0:v0 + NKV * D, :]
        wqkv_c = np.ascontiguousarray(
            np.concatenate([wk, wq[0:128], wv, wq[128:512]], axis=0).T
        ).astype(ml_dtypes.bfloat16)
        in_maps.append({
            "xt": xt_b[b],
            "wqkv": wqkv_c,
            "wo": np.ascontiguousarray(
                Wo[:, hq0 * D:(hq0 + NH) * D].T
            ).astype(ml_dtypes.bfloat16),
            "cost": cost,
            "sints": sints,
            "constsb": constsb,
        })
    return in_maps


def kernel(cos, sin, hidden_states, per_head_scale, Wqkv, Wo, _trace=False):
    nc = _build()
    in_maps = _prep_inputs(cos, sin, hidden_states, per_head_scale, Wqkv, Wo)
    res = run_bass_kernel_spmd(nc, in_maps, core_ids=list(range(8)), trace=_trace)
    _CACHED["last_results"] = res
    out = np.stack([
        sum(res.results[b * 4 + g]["out"].astype(np.float64) for g in range(4))
        for b in range(B)
    ]).astype(np.float32)
    return out


# revision 40
# speedup vs baseline: 1.1833x; 1.1833x over previous
"""GQA attention (RoPE, causal, per-head q-scale) on 8 TRN2 NeuronCores.

Sharding: 2-way data-parallel over batch x 4-way tensor-parallel over heads.
Core c handles batch b=c//4 and head group g=c%4 (8 q heads, 2 kv heads).
Each core computes qkv-proj -> rope -> causal attention -> partial o_proj
(over its heads' columns of Wo); the host sums the 4 partials per batch.

All scalar factors (rope_mscale, sm_scale, per_head_scale) are folded into
the Wq/Wk rows on the host. Causal masking: fully-masked column blocks are
skipped (matmul widths trimmed to the causal extent); diagonal blocks are
masked by multiplying the exp'd bf16 tile with a 0/1 lower-triangular
constant on the vector engine (the Vaug ones-columns then produce exact
softmax denominators).

dtypes: matmuls run in bf16 with f32 PSUM accumulation; rope runs in bf16
(inputs are bf16 for the scores matmul anyway); softmax normalization is
f32 (reciprocal_approx_fast). The o_proj partials are written out in bf16
(summed in f64 on the host).

Emission is software-pipelined at sk-block granularity: background quanta
(QKV(j+1) k-passes, o_proj(j-1) column blocks) are interleaved between
attention score blocks so the PE always has exp-independent work while the
scalar engine streams the exps. The j=0 QKV runs as a DMA-paced prologue:
per-k passes for all 6 output chunks are interleaved with the streaming
weight/activation loads, so real PE work starts ~2us into the kernel.

Layouts on device (partition, free):
  xt      [hid, s]        hidden^T, streamed in 512-col chunks
  wqkv    [hid, 768]      [Wq(8 heads, scaled) | Wk(2 kv, scaled) | Wv].T
  q/k^T   [d*heads, s]    head-major rows; rope applied in this layout
  scores^T[sk, sq]        per (head-pair, sk-chunk 128, sq-chunk 512) in
                          PSUM; band blocks compacted: head A in cols
                          [off:512], head B in [512:1024-off]
  exp^T   [sk, sq]        SBUF bf16, fed as matmul rhs
  Vaug    [sk, 128]       V rows (0:64) + 64 ones cols; PV matmul output
                          rows 64:128 then hold the softmax denominators
                          already broadcast over 64 partitions
  out^T   [2d, sq]        PSUM accumulator per (head, sq-chunk)
  attn^T  [o(=2 heads), s] normalized bf16, lhsT for o_proj
  out     [s, hid_out]    partial o_proj result (bf16), one per core
"""

import sys, os
from collections import deque

for _p in ("/opt/trn_rl_repo", "/root/.axon_site/_ro/trn_rl_repo"):
    if os.path.isdir(_p) and _p not in sys.path:
        sys.path.insert(0, _p)

import numpy as np

import concourse.bass as bass
import concourse.mybir as mybir
import concourse.tile as tile
from concourse import bacc
from concourse.bass_utils import run_bass_kernel_spmd

F32 = mybir.dt.float32
BF16 = mybir.dt.bfloat16
AF = mybir.ActivationFunctionType

B, S, HID = 2, 2048, 2048
H, K, D = 32, 8, 64
G = H // K
ROPE_MSCALE = 1.2
SM_SCALE = 1.0 / (D ** 0.5)

NH = 8           # q heads per core
NKV = 2          # kv heads per core
NPAIR = 4        # q head pairs per core
QO = NH * D      # 512 q rows
NK = HID // 128  # 16 contraction chunks
SQW = 512        # sq / xt chunk width
NJ = S // SQW    # 4 chunks
NSK = S // 128   # 16 sk chunks

_CACHED = {}


def _build():
    if "nc" in _CACHED:
        return _CACHED["nc"]

    nc = bacc.Bacc(None)

    xt_d = nc.declare_dram_parameter("xt", [HID, S], BF16, isOutput=False)
    wqkv_d = nc.declare_dram_parameter("wqkv", [HID, 768], BF16, isOutput=False)
    wo_d = nc.declare_dram_parameter("wo", [QO, HID], BF16, isOutput=False)
    cost_d = nc.declare_dram_parameter("cost", [128, S], BF16, isOutput=False)
    sints_d = nc.declare_dram_parameter("sints", [64, S], BF16, isOutput=False)
    constsb_d = nc.declare_dram_parameter("constsb", [128, 256], BF16, isOutput=False)
    out_d = nc.declare_dram_parameter("out", [S, HID], BF16, isOutput=True)

    with tile.TileContext(nc) as tc:
        # ---------- long-lived pools ----------
        with (
            tc.tile_pool(name="consts", bufs=1) as consts_pool,
            tc.tile_pool(name="ktv", bufs=1) as ktv_pool,
            tc.tile_pool(name="qrope", bufs=10) as qrope_pool,
            tc.tile_pool(name="expt", bufs=8) as expt_pool,
            tc.tile_pool(name="attnt", bufs=8) as attnt_pool,
            tc.tile_pool(name="inv", bufs=2) as inv_pool,
            tc.tile_pool(name="wo", bufs=1) as wo_pool,
            tc.tile_pool(name="ost", bufs=4) as ost_pool,
            tc.tile_pool(name="wq", bufs=1) as wq_pool,
            tc.tile_pool(name="xt", bufs=12) as xt_pool,
            tc.tile_pool(name="cs", bufs=1) as cs_pool,
            tc.tile_pool(name="rtmp", bufs=2) as rtmp_pool,
            tc.tile_pool(name="psc", bufs=2, space="PSUM") as psc_pool,
            tc.tile_pool(name="pout2", bufs=2, space="PSUM") as pout2_pool,
            tc.tile_pool(name="pqkv", bufs=2, space="PSUM") as pqkv_pool,
        ):
            xt_r = xt_d.rearrange("(kc p) s -> p kc s", p=128)
            xt_tiles = {}

            def load_xt_kq(j, kq):
                if j not in xt_tiles:
                    xt_tiles[j] = [None] * 4
                tt = xt_pool.tile([128, 4, SQW], BF16, tag="xt", name="xtt")
                nc.sync.dma_start(
                    out=tt,
                    in_=xt_r[:, kq * 4:(kq + 1) * 4, j * SQW:(j + 1) * SQW],
                )
                xt_tiles[j][kq] = tt

            def load_xt(j):
                for kq in range(4):
                    load_xt_kq(j, kq)

            # tiny HAM warm-up: a few dummy matmuls ramp the PE clock while
            # the first weight/activation slices stream in.
            dummy = cs_pool.tile([128, 512], BF16, tag="dummy", name="dummy")
            nc.vector.memset(dummy, 1.0)
            pwarm = pout2_pool.tile([128, 512], F32, tag="p2", name="pwarm")
            for _ in range(4):
                nc.tensor.matmul(
                    pwarm, dummy[:, 0:128], dummy, start=True, stop=True,
                    skip_group_check=True,
                )

            # interleaved DMA issue: the host lays wqkv out as
            # [Wk | Wq-pair0 | Wv | Wq-pairs1-3] so the prologue-critical
            # first 384 cols stream as one contiguous run per k-slice,
            # arriving just before the prologue's k-pass needs them; the
            # remaining q-pair weights stream second.
            wqt = wq_pool.tile([128, NK, 768], BF16, name="wqt")
            def wq_col(m):
                # host layout [Wk | Wq-pair0 | Wv | Wq-pair1..3]
                if m == 4:
                    return 0        # k-proj
                if m == 0:
                    return 128      # q pair 0
                if m == 5:
                    return 256      # v-proj
                return 256 + 128 * m  # q pairs 1..3 -> 384/512/640

            wq_r = wqkv_d.rearrange("(kc p) c -> p kc c", p=128)
            costd = cs_pool.tile([128, S], BF16, tag="cost", name="costd")
            sints = cs_pool.tile([64, S], BF16, tag="sints", name="sints")
            constsb = consts_pool.tile([128, 256], BF16, name="constsb")

            # the sync queue's in-flight cap self-throttles: transfers land
            # in issue order at fabric rate, so the critical stream (k, V,
            # q-pairs 0-1, rope tables) is issued first in consumption
            # order with chunked transfers (per-transfer trigger
            # instructions serialize on the issuing engine, so fewer,
            # bigger DMAs issue faster), and the secondary stream strictly
            # after
            nc.sync.dma_start(out=constsb, in_=constsb_d[:, :])
            load_xt_kq(0, 0)
            nc.sync.dma_start(out=wqt[:, 0:4, 0:384], in_=wq_r[:, 0:4, 0:384])
            load_xt_kq(0, 1)
            nc.sync.dma_start(out=wqt[:, 4:8, 0:384], in_=wq_r[:, 4:8, 0:384])
            load_xt_kq(0, 2)
            nc.sync.dma_start(out=wqt[:, 8:12, 0:384], in_=wq_r[:, 8:12, 0:384])
            load_xt_kq(0, 3)
            nc.sync.dma_start(out=wqt[:, 12:16, 0:384], in_=wq_r[:, 12:16, 0:384])
            nc.sync.dma_start(out=sints, in_=sints_d[:, :])
            nc.sync.dma_start(out=costd, in_=cost_d[:, :])
            # q-pair 1 weights, then the long tail: q-pairs 2-3, xt(1), Wo
            nc.sync.dma_start(out=wqt[:, :, 384:512], in_=wq_r[:, :, 384:512])
            nc.sync.dma_start(out=wqt[:, :, 512:768], in_=wq_r[:, :, 512:768])
            wot = wo_pool.tile([128, NPAIR, HID], BF16, name="wot")
            wo_r = wo_d.rearrange("(m p) h -> p m h", p=128)
            xt_tiles[1] = [None] * 4
            for kq in range(4):
                tt = xt_pool.tile([128, 4, SQW], BF16, tag="xt", name="xtt")
                nc.sync.dma_start(
                    out=tt, in_=xt_r[:, kq * 4:(kq + 1) * 4, SQW:2 * SQW]
                )
                xt_tiles[1][kq] = tt
            nc.sync.dma_start(out=wot, in_=wo_r[:, :, :])

            mask01 = constsb[:, 0:128]      # 1.0 where sq >= sk (in-block)
            ones_colb = constsb[:, 128:129]

            kt_aa = ktv_pool.tile([128, S], BF16, tag="ktaa", name="ktaa")
            kt_bb = ktv_pool.tile([128, S], BF16, tag="ktbb", name="ktbb")
            # Vaug: cols 0:64 = V, cols 64:128 = 1.0 (sums -> rows 64:128)
            vaug = [
                ktv_pool.tile([128, NSK, 128], BF16, tag=f"vaug{i}", name=f"vaug{i}")
                for i in range(NKV)
            ]

            qrope = {}   # (m, j) -> tile [128, SQW] bf16
            attnt = {}   # (m, j) -> tile [128, SQW] bf16

            def rope(psum_q, j, dst):
                """RoPE a [128, SQW] projected chunk (2 heads) into dst.
                q' = q*cos + swap_halves(q)*sin_signed.  The five multiplies
                read PSUM directly (mixed-space ops may differ in base
                partition; SBUF-SBUF ops may not) and emit bf16, so the
                final add runs as a 2-byte SBUF op in the DVE fast mode."""
                c0, c1 = j * SQW, (j + 1) * SQW
                t2 = rtmp_pool.tile([128, SQW], BF16, tag="t2", name="t2")
                for base in (0, 64):
                    nc.vector.tensor_mul(
                        t2[base:base + 32, :], psum_q[base + 32:base + 64, :],
                        sints[0:32, c0:c1],
                    )
                    nc.vector.tensor_mul(
                        t2[base + 32:base + 64, :], psum_q[base:base + 32, :],
                        sints[32:64, c0:c1],
                    )
                t4 = rtmp_pool.tile([128, SQW], BF16, tag="t4", name="t4")
                nc.vector.tensor_mul(t4, psum_q, costd[:, c0:c1])
                nc.vector.tensor_add(dst, t2, t4)

            def finish_k_chunk(pq, j):
                kro = rtmp_pool.tile([128, SQW], BF16, tag="kro", name="kro")
                rope(pq, j, kro)
                c0, c1 = j * SQW, (j + 1) * SQW
                # partition-aligned copies on the scalar engine, shifted
                # ones on the vector engine: halves the serialized chain
                nc.scalar.copy(kt_aa[0:64, c0:c1], kro[0:64, :])
                nc.vector.tensor_copy(kt_aa[64:128, c0:c1], kro[0:64, :])
                nc.vector.tensor_copy(kt_bb[0:64, c0:c1], kro[64:128, :])
                nc.scalar.copy(kt_bb[64:128, c0:c1], kro[64:128, :])

            # ---------- background work quanta ----------
            bg = deque()  # (tag, fn)
            bg_state = {"credit": 0.0, "rate": 0.0}

            def bg_set_rate(blocks):
                bg_state["rate"] = len(bg) / max(blocks, 1)
                bg_state["credit"] = 0.0

            def bg_tick():
                bg_state["credit"] += bg_state["rate"]
                while bg_state["credit"] >= 1.0 and bg:
                    bg.popleft()[1]()
                    bg_state["credit"] -= 1.0

            def bg_flush():
                while bg:
                    bg.popleft()[1]()

            def bg_drain_qkv():
                while any(t == "qkv" for t, _ in bg):
                    bg.popleft()[1]()

            def qkv_quanta(j, m):
                """qkv chunk m of sq-chunk j as 4 quanta of 4 k-passes."""
                state = {}

                def passes(k0):
                    def f():
                        if k0 == 0:
                            state["pq"] = pqkv_pool.tile(
                                [128, SQW], F32, tag="qkv", name="pqkv"
                            )
                        pq = state["pq"]
                        wc = wq_col(m)
                        for k in range(k0, k0 + 4):
                            nc.tensor.matmul(
                                pq,
                                wqt[:, k, wc:wc + 128],
                                xt_tiles[j][k // 4][:, k % 4, :],
                                start=(k == 0),
                                stop=(k == NK - 1),
                            )
                        if k0 == 12:
                            if m < NPAIR:
                                qrope[(m, j)] = qrope_pool.tile(
                                    [128, SQW], BF16, tag="qr", name="qr"
                                )
                                rope(pq, j, qrope[(m, j)])
                            else:
                                finish_k_chunk(pq, j)
                    return f

                return [("qkv", passes(0)), ("qkv", passes(4)),
                        ("qkv", passes(8)), ("qkv", passes(12))]

            def v_quanta(j, copy_eng=None, h2s=(0, 1, 2, 3)):
                """V chunk of sq-chunk j as quanta (one per 128 tokens)."""
                def one(h2):
                    def f():
                        sk = (j * SQW) // 128 + h2
                        pv = pqkv_pool.tile(
                            [128, 128], F32, tag="qkv", name="pv"
                        )
                        for k in range(NK):
                            nc.tensor.matmul(
                                pv,
                                xt_tiles[j][k // 4][:, k % 4, h2 * 128:(h2 + 1) * 128],
                                wqt[:, k, 256:384],
                                start=(k == 0),
                                stop=(k == NK - 1),
                            )
                        for i in range(NKV):
                            if copy_eng is nc.scalar:
                                nc.scalar.copy(
                                    vaug[i][:, sk, 0:64],
                                    pv[:, i * 64:(i + 1) * 64],
                                )
                            else:
                                nc.vector.tensor_copy(
                                    vaug[i][:, sk, 0:64],
                                    pv[:, i * 64:(i + 1) * 64],
                                )
                    return f

                return [("qkv", one(h2)) for h2 in h2s]

            _ocp = {"n": 0}

            def oproj_quanta(j, pools=None):
                """o_proj of sq-chunk j: 16 quanta (one per 128 tokens x
                512 hid cols).  Copies rotate over vector/scalar/gpsimd;
                the final chunk also rotates psum pools so the tail is not
                serialized on two banks."""
                pools = pools or [(pqkv_pool, "qkv")]

                ots = {}

                def one(sc, hc):
                    def f():
                        i = _ocp["n"]
                        _ocp["n"] += 1
                        pool, ptag = pools[i % len(pools)]
                        po = pool.tile([128, 512], F32, tag=ptag, name="po")
                        for m in range(NPAIR):
                            nc.tensor.matmul(
                                po,
                                attnt[(m, j)][:, sc * 128:(sc + 1) * 128],
                                wot[:, m, hc * 512:(hc + 1) * 512],
                                start=(m == 0),
                                stop=(m == NPAIR - 1),
                            )
                        # gather the four 512-col blocks of this 128-token
                        # group into one tile: a single contiguous-row DMA
                        # moves 4KB/row instead of 4 fragmented 1KB writes
                        if hc == 0:
                            ots[sc] = ost_pool.tile(
                                [128, HID], BF16, tag="ot", name="ot"
                            )
                        ot = ots[sc]
                        if i % 2 == 0:
                            nc.vector.tensor_copy(
                                ot[:, hc * 512:(hc + 1) * 512], po
                            )
                        else:
                            nc.scalar.copy(ot[:, hc * 512:(hc + 1) * 512], po)
                        if hc == HID // 512 - 1:
                            r0 = j * SQW + sc * 128
                            nc.gpsimd.dma_start(
                                out=out_d[r0:r0 + 128, :], in_=ots.pop(sc)
                            )
                        if sc == SQW // 128 - 1 and hc == HID // 512 - 1:
                            for m in range(NPAIR):
                                attnt.pop((m, j))
                    return f

                return [("oproj", one(sc, hc)) for sc in range(4) for hc in range(4)]

            _done_pairs = set()
            _fin = {"f": None}  # deferred tail of the previous pair

            def attention_pair(j, m):
                if (j, m) in _done_pairs:
                    return
                _done_pairs.add((j, m))
                nsk = 4 * (j + 1)
                kt = kt_aa if m < 2 else kt_bb
                va = vaug[m // 2]
                qr = qrope.pop((m, j))
                p2 = {}
                for hb in (0, 64):  # head A at 0, head B at 64
                    p2[hb] = pout2_pool.tile(
                        [128, SQW], F32, tag="p2", name="p2"
                    )
                pend = []  # staged (exp tile, sk)

                def pv_step():
                    et2, psk = pend.pop(0)
                    poff = (psk - 4 * j) * 128 if psk >= 4 * j else 0
                    nc.tensor.matmul(
                        p2[0][:, poff:SQW],
                        va[:, psk, :],
                        et2[:, poff:SQW],
                        start=(psk == 0),
                        stop=(psk == nsk - 1),
                        skip_group_check=True,
                    )
                    nc.tensor.matmul(
                        p2[64][:, poff:SQW],
                        va[:, psk, :],
                        et2[:, SQW:2 * SQW - poff],
                        start=(psk == 0),
                        stop=(psk == nsk - 1),
                        skip_group_check=True,
                    )

                for sk in range(nsk):
                    # both heads' scores in one 2-bank tile; band blocks
                    # compacted so the valid region is contiguous:
                    # head A [off:512], head B [512:1024-off]
                    p1 = psc_pool.tile([128, 2 * SQW], F32, tag="sc", name="sc")
                    band = sk >= 4 * j
                    off = (sk - 4 * j) * 128 if band else 0
                    nc.tensor.matmul(
                        p1[:, off:SQW],
                        kt[0:64, sk * 128:(sk + 1) * 128],
                        qr[0:64, off:SQW],
                        start=True, stop=True, skip_group_check=True,
                    )
                    nc.tensor.matmul(
                        p1[:, SQW:2 * SQW - off],
                        kt[64:128, sk * 128:(sk + 1) * 128],
                        qr[64:128, off:SQW],
                        start=True, stop=True, skip_group_check=True,
                    )
                    et = expt_pool.tile(
                        [128, 2 * SQW], BF16, tag="et", name="et"
                    )
                    nc.scalar.activation(
                        et[:, off:2 * SQW - off], p1[:, off:2 * SQW - off],
                        AF.Exp,
                    )
                    if band:
                        # causal mask on the diagonal 128x128 sub-blocks,
                        # on the (otherwise idle) gpsimd engine so the
                        # exp->mask->PV chain skips the deep DVE queue
                        nc.gpsimd.tensor_mul(
                            et[:, off:off + 128], et[:, off:off + 128], mask01
                        )
                        nc.gpsimd.tensor_mul(
                            et[:, SQW:SQW + 128], et[:, SQW:SQW + 128], mask01
                        )
                    pend.append((et, sk))
                    if sk == 1 and _fin["f"] is not None:
                        # emit the previous pair's deferred tail now: its
                        # last exps drain while this pair's first scores
                        # run, instead of stalling the PE at the boundary
                        _fin["f"]()
                        _fin["f"] = None
                    bg_tick()
                    while len(pend) > 3:
                        pv_step()

                def finish():
                    while pend:
                        pv_step()
                    # normalize: attnT = out^T * (1/sums); sums come out
                    # of the PV matmul pre-broadcast in psum rows 64:128
                    at = attnt_pool.tile([128, SQW], BF16, tag="at", name="at")
                    for hb in (0, 64):
                        sums = inv_pool.tile(
                            [64, SQW], F32, tag="sums", name="sums"
                        )
                        nc.vector.tensor_copy(sums, p2[hb][64:128, :])
                        invb = inv_pool.tile(
                            [64, SQW], F32, tag="invb", name="invb"
                        )
                        nc.vector.reciprocal_approx_fast(out=invb, in_=sums)
                        nc.vector.tensor_mul(
                            at[hb:hb + 64, :], p2[hb][0:64, :], invb
                        )
                    attnt[(m, j)] = at

                if _fin["f"] is not None:
                    # nsk < 2 never happens, but guard double-defer anyway
                    _fin["f"]()
                _fin["f"] = finish

            # ---------- j=0 QKV prologue, DMA-paced ----------
            # Critical path only: k-chunk + q-pair 0 interleaved per-k (the
            # PE consumes weight slices as they stream in), then V with one
            # psum accumulation group open at a time.  q-pairs 1-3 are
            # seeded as background quanta and run during attention(0, *).
            pk0 = pqkv_pool.tile([128, SQW], F32, tag="qkv", name="pk0")
            pm0 = pqkv_pool.tile([128, SQW], F32, tag="qkv", name="pm0")
            # V token-groups 0-1 ride the (idle until attention) psc slots
            # so they stream through the k-loop with everything else
            pv01 = [
                psc_pool.tile([128, 2 * SQW], F32, tag="sc", name=f"pvh{h2}")
                for h2 in (0, 1)
            ]
            for k in range(NK):
                xt_t0 = xt_tiles[0][k // 4][:, k % 4, :]
                st, sp = (k == 0), (k == NK - 1)
                nc.tensor.matmul(pk0, wqt[:, k, 0:128], xt_t0, start=st, stop=sp)
                nc.tensor.matmul(pm0, wqt[:, k, 128:256], xt_t0, start=st, stop=sp)
                for h2 in (0, 1):
                    nc.tensor.matmul(
                        pv01[h2][:, 0:128],
                        xt_tiles[0][k // 4][:, k % 4, h2 * 128:(h2 + 1) * 128],
                        wqt[:, k, 256:384],
                        start=st, stop=sp,
                    )
            finish_k_chunk(pk0, 0)
            qrope[(0, 0)] = qrope_pool.tile([128, SQW], BF16, tag="qr", name="qr")
            rope(pm0, 0, qrope[(0, 0)])
            # ones-fill emitted here so it never blocks the rope chain in
            # the DVE queue (constsb is loaded first and long since landed)
            for i in range(NKV):
                nc.vector.tensor_copy(
                    vaug[i][:, :, 64:128],
                    ones_colb[:, None, :].broadcast_to([128, NSK, 64]),
                )
            for h2 in (0, 1):
                for i in range(NKV):
                    nc.scalar.copy(
                        vaug[i][:, h2, 0:64],
                        pv01[h2][:, i * 64:(i + 1) * 64],
                    )
            for _, f in v_quanta(0, copy_eng=nc.scalar, h2s=(2, 3)):
                f()
            for m in range(1, NPAIR):
                bg.extend(qkv_quanta(0, m))
            # q-pairs 1-2 keep the PE fed while the rope/copy chain
            # drains on DVE/ACT; pair 3 stays background (its weights ride
            # the late secondary DMA stream — pre-emitting it stalls the
            # in-order PE queue)
            for _ in range(8):
                bg.popleft()[1]()

            def ensure_qrope(j, m):
                if (j, m) in _done_pairs:
                    return
                while (m, j) not in qrope and bg:
                    bg.popleft()[1]()

            # ---------- main loop ----------
            for j in range(NJ):
                if j > 0:
                    xt_tiles.pop(j - 1)
                if j + 2 < NJ:
                    load_xt(j + 2)
                if j + 1 < NJ:
                    for m in range(5):
                        bg.extend(qkv_quanta(j + 1, m))
                    bg.extend(v_quanta(j + 1))
                if j > 0:
                    bg.extend(oproj_quanta(j - 1))
                nsk = 4 * (j + 1)
                blocks = 4 * nsk + (2 * 4 * NJ if j == NJ - 2 else 0)
                if j == NJ - 1:
                    blocks = 2 * nsk
                bg_set_rate(blocks)
                for m in range(NPAIR):
                    ensure_qrope(j, m)
                    attention_pair(j, m)
                if j == NJ - 2:
                    # pull two of the last chunk's head-pairs forward so the
                    # tail window keeps the PE fed; their kt/vaug/qrope
                    # writes must be emitted first
                    bg_drain_qkv()
                    ensure_qrope(NJ - 1, 0)
                    attention_pair(NJ - 1, 0)
                    ensure_qrope(NJ - 1, 1)
                    attention_pair(NJ - 1, 1)
                bg_flush()
            if _fin["f"] is not None:
                _fin["f"]()
                _fin["f"] = None
            for _, f in oproj_quanta(
                NJ - 1,
                pools=[(pqkv_pool, "qkv"), (psc_pool, "sc"), (pout2_pool, "p2")],
            ):
                f()

    nc.finalize()
    _CACHED["nc"] = nc
    return nc


def _prep_inputs(cos, sin, hidden_states, per_head_scale, Wqkv, Wo):
    """Build the 8 per-core input maps (host-side, free)."""
    import ml_dtypes
    cos = np.asarray(cos, np.float32)
    sin = np.asarray(sin, np.float32)
    hs = np.asarray(hidden_states, np.float32)
    phs = np.asarray(per_head_scale, np.float32)
    Wqkv = np.asarray(Wqkv, np.float32)
    Wo = np.asarray(Wo, np.float32)

    cost = np.ascontiguousarray(np.vstack([cos.T, cos.T])).astype(ml_dtypes.bfloat16)
    st = sin.T.copy()
    st[0:32] *= -1.0
    sints = np.ascontiguousarray(st).astype(ml_dtypes.bfloat16)

    mask01 = np.zeros((128, 128), np.float32)
    for p in range(128):
        mask01[p, p:] = 1.0
    pad = np.zeros((128, 128), np.float32)
    pad[:, 0] = 1.0
    constsb = np.ascontiguousarray(
        np.concatenate([mask01, pad], axis=1)
    ).astype(ml_dtypes.bfloat16)

    xt_b = [np.ascontiguousarray(hs[b].T).astype(ml_dtypes.bfloat16) for b in range(B)]

    in_maps = []
    for c in range(8):
        b, g = c // 4, c % 4
        hq0 = NH * g
        wq = Wqkv[hq0 * D:(hq0 + NH) * D, :].copy()
        for h in range(NH):
            wq[h * D:(h + 1) * D] *= (
                ROPE_MSCALE * SM_SCALE * phs[b, hq0 + h]
            )
        kv0 = H * D + NKV * g * D
        wk = Wqkv[kv0:kv0 + NKV * D, :] * ROPE_MSCALE
        v0 = (H + K) * D + NKV * g * D
        wv = Wqkv[v0:v0 + NKV * D, :]
        wqkv_c = np.ascontiguousarray(
            np.concatenate([wk, wq[0:128], wv, wq[128:512]], axis=0).T
        ).astype(ml_dtypes.bfloat16)
        in_maps.append({
            "xt": xt_b[b],
            "wqkv": wqkv_c,
            "wo": np.ascontiguousarray(
                Wo[:, hq0 * D:(hq0 + NH) * D].T
            ).astype(ml_dtypes.bfloat16),
            "cost": cost,
            "sints": sints,
            "constsb": constsb,
        })
    return in_maps


def kernel(cos, sin, hidden_states, per_head_scale, Wqkv, Wo, _trace=False):
    nc = _build()
    in_maps = _prep_inputs(cos, sin, hidden_states, per_head_scale, Wqkv, Wo)
    res = run_bass_kernel_spmd(nc, in_maps, core_ids=list(range(8)), trace=_trace)
    _CACHED["last_results"] = res
    out = np.stack([
        sum(res.results[b * 4 + g]["out"].astype(np.float64) for g in range(4))
        for b in range(B)
    ]).astype(np.float32)
    return out
